# revision 12
# baseline (speedup 1.0000x reference)
"""Trainium2 Bass kernel for nn_Caption (LSTM caption decoder).

Distribution: pure data-parallel over batch (128 -> 8 cores x 16), no
collectives. Per core: x0 projection GEMM, embedding gather, input-gate
GEMM, 40-step LSTM recurrence, vocab GEMM [640,512]@[512,10240].

Key layout ideas:
- Gates GEMM is col-tiled 4x via tile_position: PE col-strip j computes,
  for all 4 gate types, the hidden-dim subrange 128j:128j+128 (weight
  columns host-permuted to [i_j | f_j | o_j | g_j] per strip). The four
  strips' matmuls run concurrently on the PE (distinct col groups), and
  every LSTM elementwise op is partition-aligned across strips, running
  on [112, 128]-shaped tiles (4x lane packing).
- Each step's PSUM bank is opened by one full-bank "dummy" matmul with
  start=True that simultaneously injects the combined gate bias
  (strip-selector lhsT x bias rhs). xg is injected via identity matmuls
  (one per strip, concurrent).
- h is produced per-strip as h[b, 128j+c] and transposed (4 PE
  transposes/step) straight into the stationary hiddensT buffer consumed
  by the vocab GEMM (hidT tiles stationary, W_out streamed at N=512).
- Output is written bf16 and upcast on host.
"""
import sys

sys.path.insert(0, "/opt/trn_rl_repo")

import numpy as np
import ml_dtypes

import concourse.bass as bass
import concourse.tile as tile
from concourse import bacc, mybir
from concourse.bass_utils import run_bass_kernel_spmd

BF = mybir.dt.bfloat16
F32 = mybir.dt.float32
I32 = mybir.dt.int32
bfnp = ml_dtypes.bfloat16
ACTF = mybir.ActivationFunctionType

B, F, E, H, V, T = 128, 1536, 512, 512, 10000, 40
NCORES = 8
BC = B // NCORES          # 16 batch rows per core
NB = T * BC               # 640 (t,b) columns, t-major
G4 = 4 * H                # 2048 gate dims
VP = 10240                # padded vocab
NVC = VP // 512           # 20 vocab 512-chunks
NTW = 5                   # t-windows of 8 steps (128 tb-cols each)
NMT = NB // 128           # 5 xg m-tiles

_CACHE = {}


def _build():
    if "nc" in _CACHE:
        return _CACHE["nc"]
    nc = bacc.Bacc("TRN2", target_bir_lowering=False, debug=False,
                   num_devices=NCORES)

    featT_d = nc.dram_tensor("featT", [F, BC], BF, kind="ExternalInput")
    idx_d = nc.dram_tensor("idx", [NB, 1], I32, kind="ExternalInput")
    emb_d = nc.dram_tensor("embt", [V, E], BF, kind="ExternalInput")
    WinT_d = nc.dram_tensor("WinT", [128, 12, E], BF, kind="ExternalInput")
    WihT_d = nc.dram_tensor("WihT", [128, 4, G4], BF, kind="ExternalInput")
    WhhT_d = nc.dram_tensor("WhhT", [128, 4, G4], BF, kind="ExternalInput")
    bias4_d = nc.dram_tensor("bias4", [32, 512], BF, kind="ExternalInput")
    stripsel_d = nc.dram_tensor("stripsel", [32, 112], BF,
                                kind="ExternalInput")
    bin_d = nc.dram_tensor("bin", [E], F32, kind="ExternalInput")
    ident_d = nc.dram_tensor("ident", [128, 128], BF, kind="ExternalInput")
    wout_d = nc.dram_tensor("wout", [128, 4, NVC, 512], BF,
                            kind="ExternalInput")
    out_d = nc.dram_tensor("out_q", [NTW, NVC, 128, 512], BF,
                           kind="ExternalOutput")

    with tile.TileContext(nc) as tc:
        with (
            tc.tile_pool(name="big", bufs=1) as big,
            tc.tile_pool(name="state", bufs=2) as state,
            tc.tile_pool(name="work", bufs=3) as work,
            tc.tile_pool(name="vout", bufs=4) as voutp,
        ):
            # ---- constant loads ----
            wout_sb = big.tile([128, 4, NVC, 512], BF, tag="wout")
            nc.sync.dma_start(wout_sb[:], wout_d.ap())
            WhhT_sb = big.tile([128, 4, G4], BF, tag="whh")
            nc.sync.dma_start(WhhT_sb[:], WhhT_d.ap())
            WihT_sb = big.tile([128, 4, G4], BF, tag="wih")
            nc.sync.dma_start(WihT_sb[:], WihT_d.ap())
            WinT_sb = big.tile([128, 12, E], BF, tag="win")
            nc.sync.dma_start(WinT_sb[:], WinT_d.ap())
            idx_sb = big.tile([128, NB // 128, 1], I32, tag="idx")
            nc.sync.dma_start(
                idx_sb[:], idx_d.ap().rearrange("(j p) o -> p j o", p=128))
            identb = big.tile([128, 128], BF, tag="ident")
            nc.sync.dma_start(identb[:], ident_d.ap())
            featT_sb = big.tile([128, 12, BC], BF, tag="feat")
            nc.sync.dma_start(
                featT_sb[:], featT_d.ap().rearrange("(k p) b -> p k b", p=128))
            bias4 = big.tile([32, 512], BF, tag="bias4")
            nc.sync.dma_start(bias4[:], bias4_d.ap())
            stripsel = big.tile([32, 112], BF, tag="stripsel")
            nc.sync.dma_start(stripsel[:], stripsel_d.ap())
            bin_sb = big.tile([128, 4], F32, tag="bin")
            nc.sync.dma_start(
                bin_sb[:], bin_d.ap().rearrange("(k p) -> p k", p=128))

            seqT = big.tile([128, 4, NB], BF, tag="seqT")
            xg_sb = big.tile([128, NMT, G4], BF, tag="xg")
            hidT = big.tile([128, 4, T, BC], BF, tag="hidT")
            c_st = big.tile([112, 128], F32, tag="c")
            nc.vector.memset(c_st[:], 0.0)
            zeros1 = big.tile([128, 1], BF, tag="z1")
            nc.vector.memset(zeros1[:], 0.0)

            # ---- embedding gather -> seqT (transposed via PE) ----
            with tc.tile_pool(name="psA", bufs=3, space="PSUM") as psA:
                for j in range(NB // 128):
                    gt = work.tile([128, E], BF, tag="gather")
                    nc.gpsimd.indirect_dma_start(
                        out=gt[:], out_offset=None, in_=emb_d.ap(),
                        in_offset=bass.IndirectOffsetOnAxis(
                            ap=idx_sb[:, j, :], axis=0))
                    for e in range(4):
                        pst = psA.tile([128, 128], BF, space="PSUM", tag="tr")
                        nc.tensor.transpose(
                            pst[:], gt[:, e * 128:(e + 1) * 128], identb[:])
                        nc.scalar.copy(
                            seqT[:, e, j * 128:(j + 1) * 128], pst[:])

                # ---- x0T = W_inT.T @ featT + b_in -> seqT[:, :, 0:BC] ----
                for m in range(4):
                    ps = psA.tile([128, BC], F32, space="PSUM", tag="x0")
                    for k in range(12):
                        nc.tensor.matmul(
                            ps[:], lhsT=WinT_sb[:, k, m * 128:(m + 1) * 128],
                            rhs=featT_sb[:, k, :],
                            start=(k == 0), stop=(k == 11))
                    nc.scalar.activation(
                        seqT[:, m, 0:BC], ps[:], ACTF.Identity,
                        bias=bin_sb[:, m:m + 1])

            # ---- pools for the main phase ----
            gates_ps = tc.tile_pool(name="psG", bufs=2, space="PSUM")
            htr_ps = tc.tile_pool(name="psH", bufs=2, space="PSUM")
            xg_ps = tc.tile_pool(name="psX", bufs=2, space="PSUM")
            voc_ps = tc.tile_pool(name="psV", bufs=2, space="PSUM")
            gpsum = gates_ps.__enter__()
            hpsum = htr_ps.__enter__()
            xpsum = xg_ps.__enter__()
            vpsum = voc_ps.__enter__()

            # ---- filler quanta ----
            nxgq = [0]

            def emit_xg_quantum():
                # one (mtile, nchunk) chunk of the xg GEMM
                q = nxgq[0]
                if q >= NMT * 4:
                    return False
                mt, n = q // 4, q % 4
                nxgq[0] += 1
                ps = xpsum.tile([128, 512], F32, space="PSUM", tag="xgps")
                for k in range(4):
                    nc.tensor.matmul(
                        ps[:], lhsT=seqT[:, k, mt * 128:(mt + 1) * 128],
                        rhs=WihT_sb[:, k, n * 512:(n + 1) * 512],
                        start=(k == 0), stop=(k == 3))
                if (q % 2) == 0:
                    nc.scalar.copy(xg_sb[:, mt, n * 512:(n + 1) * 512], ps[:])
                else:
                    nc.vector.tensor_copy(
                        xg_sb[:, mt, n * 512:(n + 1) * 512], ps[:])
                return True

            nvu = [0]

            def emit_vocab_unit():
                u = nvu[0]
                if u >= NTW * NVC:
                    return False
                m, n = u // NVC, u % NVC
                nvu[0] += 1
                vps = vpsum.tile([128, 512], F32, space="PSUM", tag="vps")
                for k in range(4):
                    nc.tensor.matmul(
                        vps[:], lhsT=hidT[:, k, 8 * m:8 * m + 8, :],
                        rhs=wout_sb[:, k, n, :],
                        start=(k == 0), stop=(k == 3))
                vo = voutp.tile([128, 512], BF, tag="vo")
                if (u % 2) == 0:
                    nc.scalar.copy(vo[:], vps[:])
                else:
                    nc.vector.tensor_copy(vo[:], vps[:])
                nc.sync.dma_start(out_d.ap()[m, n], vo[:])
                return True

            # ---- gate-psum opener: bias dummy + xg injects for step t ----
            def open_gates(t):
                gp = gpsum.tile([128, 512], F32, space="PSUM", tag="gates",
                                name=f"gp{t}")
                mt, po = (t * BC) // 128, (t * BC) % 128
                po32 = po - po % 32
                # bank-clear + bias inject: out[32j+r, c] = bias4[j, c]
                nc.tensor.matmul(
                    gp[0:112, :], lhsT=stripsel[:], rhs=bias4[:],
                    start=True, stop=False)
                for j in range(4):
                    nc.tensor.matmul(
                        gp[32 * j:32 * j + 16, :],
                        lhsT=identb[po32:po32 + 32, po:po + 16],
                        rhs=xg_sb[po32:po32 + 32, mt, 512 * j:512 * j + 512],
                        start=False, stop=False,
                        tile_position=(po32, 32 * j))
                return gp

            def close_gates(gp):
                # group closer: +0, stop=True over all started partitions
                nc.tensor.matmul(
                    gp[0:112, 0:1], lhsT=identb[0:128, 0:112], rhs=zeros1[:],
                    start=False, stop=True)

            # ---- the recurrence ----
            for _ in range(4):
                emit_xg_quantum()        # m-tile 0 must precede step 0
            gp_next = open_gates(0)
            for t in range(T):
                gp = gp_next
                # recurrent matmuls (strip-rotated)
                if t > 0:
                    for k in range(4):
                        for j in range(4):
                            nc.tensor.matmul(
                                gp[32 * j:32 * j + 16, :],
                                lhsT=hidT[:, k, t - 1, :],
                                rhs=WhhT_sb[:, k, 512 * j:512 * j + 512],
                                start=False, stop=False,
                                tile_position=(0, 32 * j))
                close_gates(gp)
                # open next step's bank + filler while this step's
                # elementwise chain runs
                if t + 1 < T:
                    gp_next = open_gates(t + 1)
                if t < 16:
                    emit_xg_quantum()
                    if t >= 8:
                        emit_vocab_unit()
                elif nvu[0] < (NTW * NVC):
                    emit_vocab_unit()
                    emit_vocab_unit()

                # ---- elementwise chain, strip-aligned [112, 128] ----
                sig = state.tile([112, 384], BF, tag="sig")
                tg = state.tile([112, 128], BF, tag="tg")
                nc.scalar.activation(sig[:], gp[0:112, 0:384], ACTF.Sigmoid)
                nc.scalar.activation(tg[:], gp[0:112, 384:512], ACTF.Tanh)
                ig = state.tile([112, 128], BF, tag="ig")
                nc.vector.tensor_mul(ig[:], sig[:, 0:128], tg[:])
                if t == 0:
                    nc.vector.tensor_copy(c_st[:], ig[:])
                else:
                    fc = state.tile([112, 128], F32, tag="fc")
                    nc.vector.tensor_mul(fc[:], sig[:, 128:256], c_st[:])
                    nc.vector.tensor_add(c_st[:], fc[:], ig[:])
                tc_t = state.tile([112, 128], BF, tag="tanhc")
                nc.scalar.activation(tc_t[:], c_st[:], ACTF.Tanh)
                h_bf = state.tile([128, 128], BF, tag="h")
                nc.vector.tensor_mul(h_bf[0:112, :], sig[:, 256:384], tc_t[:])
                pst = hpsum.tile([128, 128], BF, space="PSUM", tag="htr")
                nc.tensor.transpose(pst[:], h_bf[:], identb[:])
                src = pst[:].rearrange("p (j r) -> p j r", j=4)[:, :, 0:BC]
                if (t % 2) == 0:
                    nc.scalar.copy(hidT[:, :, t, :], src)
                else:
                    nc.vector.tensor_copy(hidT[:, :, t, :], src)

            # ---- vocab tail ----
            while emit_vocab_unit():
                pass

            voc_ps.__exit__(None, None, None)
            xg_ps.__exit__(None, None, None)
            htr_ps.__exit__(None, None, None)
            gates_ps.__exit__(None, None, None)

    nc.compile()
    _CACHE["nc"] = nc
    return nc


def _prep_in_maps(features, seqs, W_in, b_in, emb, W_ih, W_hh, b_ih, b_hh,
                  W_out):
    f32 = lambda x: np.asarray(x, dtype=np.float32)
    bf = lambda x: np.ascontiguousarray(f32(x)).astype(bfnp)
    features, seqs = f32(features), np.asarray(seqs).astype(np.int64)
    # gate order [i, f, o, g], then strip-permute columns so that strip j
    # holds [i_j | f_j | o_j | g_j] (j = hidden-dim 128-subrange)
    perm = np.concatenate([np.arange(0, 2 * H), np.arange(3 * H, 4 * H),
                           np.arange(2 * H, 3 * H)])
    # strip permutation on the 2048 gate columns:
    # new col (j, q, c) <- old col q*512 + 128j + c
    new2old = np.empty(2048, np.int64)
    for j in range(4):
        for q in range(4):
            for c0 in range(128):
                new2old[j * 512 + q * 128 + c0] = q * 512 + j * 128 + c0

    WinT = np.ascontiguousarray(
        bf(f32(W_in).T).reshape(12, 128, E).transpose(1, 0, 2))
    WihT = np.ascontiguousarray(
        bf(f32(W_ih).T)[:, perm][:, new2old]
        .reshape(4, 128, G4).transpose(1, 0, 2))
    WhhT = np.ascontiguousarray(
        bf(f32(W_hh).T)[:, perm][:, new2old]
        .reshape(4, 128, G4).transpose(1, 0, 2))
    bcomb = ((f32(b_ih) + f32(b_hh))[perm])[new2old]
    bias4 = np.zeros((32, 512), dtype=bfnp)
    bias4[0:4] = bcomb.reshape(4, 512).astype(bfnp)
    stripsel = np.zeros((32, 112), dtype=bfnp)
    for j in range(4):
        stripsel[j, 32 * j:32 * j + 16] = 1.0
    emb_b = bf(emb)
    WoutT = np.zeros((H, VP), dtype=bfnp)
    WoutT[:, :V] = bf(f32(W_out).T)
    # wout[p, k, n, v] = WoutT[k*128+p, n*512+v]
    wout = np.ascontiguousarray(
        WoutT.reshape(4, 128, NVC, 512).transpose(1, 0, 2, 3))
    ident_np = np.eye(128, dtype=bfnp)
    binp = f32(b_in)

    in_maps = []
    for c in range(NCORES):
        bs = slice(c * BC, (c + 1) * BC)
        featT = bf(features[bs].T)             # [F, BC]
        idx = np.zeros((T, BC), np.int64)
        idx[1:, :] = seqs[bs].T                # t-major, t=0 block dummy
        in_maps.append({
            "featT": featT,
            "idx": idx.reshape(NB, 1).astype(np.int32),
            "embt": emb_b,
            "WinT": WinT, "WihT": WihT, "WhhT": WhhT,
            "bias4": bias4, "stripsel": stripsel, "bin": binp,
            "ident": ident_np, "wout": wout,
        })
    _CACHE["last_in_maps"] = in_maps
    return in_maps


def _assemble_core_out(oq):
    """[5, 20, 128, 512] bf16 -> [BC, T, V] f32 for one core."""
    oq = np.asarray(oq)
    lt = oq.transpose(0, 2, 1, 3).reshape(NTW, 8, BC, VP)
    return (lt.transpose(2, 0, 1, 3).reshape(BC, T, VP)[:, :, :V]
            .astype(np.float32))


def kernel(features, seqs, lengths, W_in, b_in, emb, W_ih, W_hh, b_ih, b_hh,
           W_out, b_out):
    nc = _build()
    in_maps = _prep_in_maps(features, seqs, W_in, b_in, emb, W_ih, W_hh,
                            b_ih, b_hh, W_out)
    res = run_bass_kernel_spmd(nc, in_maps, list(range(NCORES)))
    out = np.empty((B, T, V), np.float32)
    for c in range(NCORES):
        out[c * BC:(c + 1) * BC] = _assemble_core_out(res.results[c]["out_q"])
    bo = np.asarray(b_out, dtype=np.float32)
    if np.any(bo):
        out += bo
    return out


# revision 14
# speedup vs baseline: 1.1159x; 1.1159x over previous
"""Trainium2 Bass kernel for nn_Caption (LSTM caption decoder).

Distribution: pure data-parallel over batch (128 -> 8 cores x 16), no
collectives. Per core: x0 projection GEMM, embedding gather, input-gate
GEMM, 40-step LSTM recurrence, vocab GEMM [640,512]@[512,10240].

Key layout ideas:
- Gates GEMM is col-tiled 4x via tile_position: PE col-strip j computes,
  for all 4 gate types, the hidden-dim subrange 128j:128j+128 (weight
  columns host-permuted to [i_j | f_j | o_j | g_j] per strip). The four
  strips' matmuls run concurrently on the PE (distinct col groups), and
  every LSTM elementwise op is partition-aligned across strips, running
  on [112, 128]-shaped tiles (4x lane packing).
- Each step's PSUM bank is opened by one full-bank "dummy" matmul with
  start=True that simultaneously injects the combined gate bias
  (strip-selector lhsT x bias rhs). xg is injected via identity matmuls
  (one per strip, concurrent).
- h is produced per-strip as h[b, 128j+c] and transposed (4 PE
  transposes/step) straight into the stationary hiddensT buffer consumed
  by the vocab GEMM (hidT tiles stationary, W_out streamed at N=512).
- Output is written bf16 and upcast on host.
"""
import sys

sys.path.insert(0, "/opt/trn_rl_repo")

import numpy as np
import ml_dtypes

import concourse.bass as bass
import concourse.tile as tile
from concourse import bacc, mybir
from concourse.bass_utils import run_bass_kernel_spmd

BF = mybir.dt.bfloat16
F32 = mybir.dt.float32
I32 = mybir.dt.int32
bfnp = ml_dtypes.bfloat16
ACTF = mybir.ActivationFunctionType

B, F, E, H, V, T = 128, 1536, 512, 512, 10000, 40
NCORES = 8
BC = B // NCORES          # 16 batch rows per core
NB = T * BC               # 640 (t,b) columns, t-major
G4 = 4 * H                # 2048 gate dims
VP = 10240                # padded vocab
NVC = VP // 512           # 20 vocab 512-chunks
NTW = 5                   # t-windows of 8 steps (128 tb-cols each)
NMT = NB // 128           # 5 xg m-tiles

_CACHE = {}


def _build():
    if "nc" in _CACHE:
        return _CACHE["nc"]
    nc = bacc.Bacc("TRN2", target_bir_lowering=False, debug=False,
                   num_devices=NCORES)

    featT_d = nc.dram_tensor("featT", [F, BC], BF, kind="ExternalInput")
    idx_d = nc.dram_tensor("idx", [NB, 1], I32, kind="ExternalInput")
    emb_d = nc.dram_tensor("embt", [V, E], BF, kind="ExternalInput")
    WinT_d = nc.dram_tensor("WinT", [128, 12, E], BF, kind="ExternalInput")
    WihT_d = nc.dram_tensor("WihT", [128, 4, G4], BF, kind="ExternalInput")
    WhhT_d = nc.dram_tensor("WhhT", [128, 4, G4], BF, kind="ExternalInput")
    bias4_d = nc.dram_tensor("bias4", [32, 512], BF, kind="ExternalInput")
    stripsel_d = nc.dram_tensor("stripsel", [32, 112], BF,
                                kind="ExternalInput")
    bin_d = nc.dram_tensor("bin", [E], F32, kind="ExternalInput")
    ident_d = nc.dram_tensor("ident", [128, 128], BF, kind="ExternalInput")
    wout_d = nc.dram_tensor("wout", [128, 4, NVC, 512], BF,
                            kind="ExternalInput")
    out_d = nc.dram_tensor("out_q", [NTW, NVC, 128, 512], BF,
                           kind="ExternalOutput")

    with tile.TileContext(nc) as tc:
        with (
            tc.tile_pool(name="big", bufs=1) as big,
            tc.tile_pool(name="state", bufs=2) as state,
            tc.tile_pool(name="work", bufs=3) as work,
            tc.tile_pool(name="vout", bufs=4) as voutp,
        ):
            # ---- constant loads (small/critical first; wout split last) ----
            idx_sb = big.tile([128, NB // 128, 1], I32, tag="idx")
            nc.sync.dma_start(
                idx_sb[:], idx_d.ap().rearrange("(j p) o -> p j o", p=128))
            identb = big.tile([128, 128], BF, tag="ident")
            nc.sync.dma_start(identb[:], ident_d.ap())
            featT_sb = big.tile([128, 12, BC], BF, tag="feat")
            nc.sync.dma_start(
                featT_sb[:], featT_d.ap().rearrange("(k p) b -> p k b", p=128))
            bias4 = big.tile([32, 512], BF, tag="bias4")
            nc.sync.dma_start(bias4[:], bias4_d.ap())
            stripsel = big.tile([32, 112], BF, tag="stripsel")
            nc.sync.dma_start(stripsel[:], stripsel_d.ap())
            bin_sb = big.tile([128, 4], F32, tag="bin")
            nc.sync.dma_start(
                bin_sb[:], bin_d.ap().rearrange("(k p) -> p k", p=128))
            WinT_sb = big.tile([128, 12, E], BF, tag="win")
            nc.sync.dma_start(WinT_sb[:], WinT_d.ap())
            WihT_sb = big.tile([128, 4, G4], BF, tag="wih")
            nc.sync.dma_start(WihT_sb[:], WihT_d.ap())
            WhhT_sb = big.tile([128, 4, G4], BF, tag="whh")
            nc.sync.dma_start(WhhT_sb[:], WhhT_d.ap())
            wout_sb = big.tile([128, 4, NVC, 512], BF, tag="wout")
            for n in range(NVC):
                nc.sync.dma_start(wout_sb[:, :, n, :], wout_d.ap()[:, :, n, :])

            seqT = big.tile([128, 4, NB], BF, tag="seqT")
            xg_sb = big.tile([128, NMT, G4], BF, tag="xg")
            hidT = big.tile([128, 4, T, BC], BF, tag="hidT")
            c_st = big.tile([112, 128], F32, tag="c")
            nc.vector.memset(c_st[:], 0.0)
            zeros1 = big.tile([128, 1], BF, tag="z1")
            nc.vector.memset(zeros1[:], 0.0)

            # ---- embedding gather -> seqT (transposed via PE) ----
            with tc.tile_pool(name="psA", bufs=3, space="PSUM") as psA:
                for j in range(NB // 128):
                    gt = work.tile([128, E], BF, tag="gather")
                    nc.gpsimd.indirect_dma_start(
                        out=gt[:], out_offset=None, in_=emb_d.ap(),
                        in_offset=bass.IndirectOffsetOnAxis(
                            ap=idx_sb[:, j, :], axis=0))
                    for e in range(4):
                        pst = psA.tile([128, 128], BF, space="PSUM", tag="tr")
                        nc.tensor.transpose(
                            pst[:], gt[:, e * 128:(e + 1) * 128], identb[:])
                        nc.scalar.copy(
                            seqT[:, e, j * 128:(j + 1) * 128], pst[:])

                # ---- x0T = W_inT.T @ featT + b_in -> seqT[:, :, 0:BC] ----
                for m in range(4):
                    ps = psA.tile([128, BC], F32, space="PSUM", tag="x0")
                    for k in range(12):
                        nc.tensor.matmul(
                            ps[:], lhsT=WinT_sb[:, k, m * 128:(m + 1) * 128],
                            rhs=featT_sb[:, k, :],
                            start=(k == 0), stop=(k == 11))
                    nc.scalar.activation(
                        seqT[:, m, 0:BC], ps[:], ACTF.Identity,
                        bias=bin_sb[:, m:m + 1])

            # ---- pools for the main phase ----
            gates_ps = tc.tile_pool(name="psG", bufs=2, space="PSUM")
            htr_ps = tc.tile_pool(name="psH", bufs=2, space="PSUM")
            xg_ps = tc.tile_pool(name="psX", bufs=2, space="PSUM")
            voc_ps = tc.tile_pool(name="psV", bufs=2, space="PSUM")
            gpsum = gates_ps.__enter__()
            hpsum = htr_ps.__enter__()
            xpsum = xg_ps.__enter__()
            vpsum = voc_ps.__enter__()

            # ---- filler quanta ----
            nxgq = [0]

            def emit_xg_quantum():
                # one (mtile, nchunk) chunk of the xg GEMM
                q = nxgq[0]
                if q >= NMT * 4:
                    return False
                mt, n = q // 4, q % 4
                nxgq[0] += 1
                ps = xpsum.tile([128, 512], F32, space="PSUM", tag="xgps")
                for k in range(4):
                    nc.tensor.matmul(
                        ps[:], lhsT=seqT[:, k, mt * 128:(mt + 1) * 128],
                        rhs=WihT_sb[:, k, n * 512:(n + 1) * 512],
                        start=(k == 0), stop=(k == 3))
                if (q % 2) == 0:
                    nc.scalar.copy(xg_sb[:, mt, n * 512:(n + 1) * 512], ps[:])
                else:
                    nc.vector.tensor_copy(
                        xg_sb[:, mt, n * 512:(n + 1) * 512], ps[:])
                return True

            nvu = [0]

            def emit_vocab_unit():
                u = nvu[0]
                if u >= NTW * NVC:
                    return False
                m, n = u // NVC, u % NVC
                nvu[0] += 1
                vps = vpsum.tile([128, 512], F32, space="PSUM", tag="vps")
                for k in range(4):
                    nc.tensor.matmul(
                        vps[:], lhsT=hidT[:, k, 8 * m:8 * m + 8, :],
                        rhs=wout_sb[:, k, n, :],
                        start=(k == 0), stop=(k == 3))
                vo = voutp.tile([128, 512], BF, tag="vo")
                if (u % 2) == 0:
                    nc.scalar.copy(vo[:], vps[:])
                else:
                    nc.vector.tensor_copy(vo[:], vps[:])
                nc.gpsimd.dma_start(out_d.ap()[m, n], vo[:])
                return True

            # ---- gate-psum opener: bias dummy + xg injects for step t ----
            def open_gates(t):
                gp = gpsum.tile([128, 512], F32, space="PSUM", tag="gates",
                                name=f"gp{t}")
                mt, po = (t * BC) // 128, (t * BC) % 128
                po32 = po - po % 32
                # bank-clear + bias inject: out[32j+r, c] = bias4[j, c]
                nc.tensor.matmul(
                    gp[0:112, :], lhsT=stripsel[:], rhs=bias4[:],
                    start=True, stop=False)
                for j in range(4):
                    nc.tensor.matmul(
                        gp[32 * j:32 * j + 16, :],
                        lhsT=identb[po32:po32 + 32, po:po + 16],
                        rhs=xg_sb[po32:po32 + 32, mt, 512 * j:512 * j + 512],
                        start=False, stop=False,
                        tile_position=(po32, 32 * j))
                return gp

            def close_gates(gp):
                # group closer: +0, stop=True over all started partitions
                nc.tensor.matmul(
                    gp[0:112, 0:1], lhsT=identb[0:128, 0:112], rhs=zeros1[:],
                    start=False, stop=True)

            # ---- the recurrence ----
            for _ in range(4):
                emit_xg_quantum()        # m-tile 0 must precede step 0
            gp_next = open_gates(0)
            for t in range(T):
                gp = gp_next
                # recurrent matmuls (strip-rotated)
                if t > 0:
                    for k in range(4):
                        for j in range(4):
                            nc.tensor.matmul(
                                gp[32 * j:32 * j + 16, :],
                                lhsT=hidT[:, k, t - 1, :],
                                rhs=WhhT_sb[:, k, 512 * j:512 * j + 512],
                                start=False, stop=False,
                                tile_position=(0, 32 * j))
                close_gates(gp)
                # open next step's bank + filler while this step's
                # elementwise chain runs
                if t + 1 < T:
                    gp_next = open_gates(t + 1)
                if t < 16:
                    emit_xg_quantum()
                    if t >= 8:
                        emit_vocab_unit()
                elif nvu[0] < (NTW * NVC):
                    emit_vocab_unit()
                    emit_vocab_unit()

                # ---- elementwise chain, strip-aligned [112, 128] ----
                sig = state.tile([112, 384], BF, tag="sig")
                tg = state.tile([112, 128], BF, tag="tg")
                nc.scalar.activation(sig[:], gp[0:112, 0:384], ACTF.Sigmoid)
                nc.scalar.activation(tg[:], gp[0:112, 384:512], ACTF.Tanh)
                ig = state.tile([112, 128], BF, tag="ig")
                nc.vector.tensor_mul(ig[:], sig[:, 0:128], tg[:])
                if t == 0:
                    nc.vector.tensor_copy(c_st[:], ig[:])
                else:
                    fc = state.tile([112, 128], F32, tag="fc")
                    nc.vector.tensor_mul(fc[:], sig[:, 128:256], c_st[:])
                    nc.vector.tensor_add(c_st[:], fc[:], ig[:])
                tc_t = state.tile([112, 128], BF, tag="tanhc")
                nc.scalar.activation(tc_t[:], c_st[:], ACTF.Tanh)
                h_bf = state.tile([128, 128], BF, tag="h")
                nc.vector.tensor_mul(h_bf[0:112, :], sig[:, 256:384], tc_t[:])
                pst = hpsum.tile([128, 128], BF, space="PSUM", tag="htr")
                nc.tensor.transpose(pst[:], h_bf[:], identb[:])
                src = pst[:].rearrange("p (j r) -> p j r", j=4)[:, :, 0:BC]
                if (t % 2) == 0:
                    nc.scalar.copy(hidT[:, :, t, :], src)
                else:
                    nc.vector.tensor_copy(hidT[:, :, t, :], src)

            # ---- vocab tail ----
            while emit_vocab_unit():
                pass

            voc_ps.__exit__(None, None, None)
            xg_ps.__exit__(None, None, None)
            htr_ps.__exit__(None, None, None)
            gates_ps.__exit__(None, None, None)

    nc.compile()
    _CACHE["nc"] = nc
    return nc


def _prep_in_maps(features, seqs, W_in, b_in, emb, W_ih, W_hh, b_ih, b_hh,
                  W_out):
    f32 = lambda x: np.asarray(x, dtype=np.float32)
    bf = lambda x: np.ascontiguousarray(f32(x)).astype(bfnp)
    features, seqs = f32(features), np.asarray(seqs).astype(np.int64)
    # gate order [i, f, o, g], then strip-permute columns so that strip j
    # holds [i_j | f_j | o_j | g_j] (j = hidden-dim 128-subrange)
    perm = np.concatenate([np.arange(0, 2 * H), np.arange(3 * H, 4 * H),
                           np.arange(2 * H, 3 * H)])
    # strip permutation on the 2048 gate columns:
    # new col (j, q, c) <- old col q*512 + 128j + c
    new2old = np.empty(2048, np.int64)
    for j in range(4):
        for q in range(4):
            for c0 in range(128):
                new2old[j * 512 + q * 128 + c0] = q * 512 + j * 128 + c0

    WinT = np.ascontiguousarray(
        bf(f32(W_in).T).reshape(12, 128, E).transpose(1, 0, 2))
    WihT = np.ascontiguousarray(
        bf(f32(W_ih).T)[:, perm][:, new2old]
        .reshape(4, 128, G4).transpose(1, 0, 2))
    WhhT = np.ascontiguousarray(
        bf(f32(W_hh).T)[:, perm][:, new2old]
        .reshape(4, 128, G4).transpose(1, 0, 2))
    bcomb = ((f32(b_ih) + f32(b_hh))[perm])[new2old]
    bias4 = np.zeros((32, 512), dtype=bfnp)
    bias4[0:4] = bcomb.reshape(4, 512).astype(bfnp)
    stripsel = np.zeros((32, 112), dtype=bfnp)
    for j in range(4):
        stripsel[j, 32 * j:32 * j + 16] = 1.0
    emb_b = bf(emb)
    WoutT = np.zeros((H, VP), dtype=bfnp)
    WoutT[:, :V] = bf(f32(W_out).T)
    # wout[p, k, n, v] = WoutT[k*128+p, n*512+v]
    wout = np.ascontiguousarray(
        WoutT.reshape(4, 128, NVC, 512).transpose(1, 0, 2, 3))
    ident_np = np.eye(128, dtype=bfnp)
    binp = f32(b_in)

    in_maps = []
    for c in range(NCORES):
        bs = slice(c * BC, (c + 1) * BC)
        featT = bf(features[bs].T)             # [F, BC]
        idx = np.zeros((T, BC), np.int64)
        idx[1:, :] = seqs[bs].T                # t-major, t=0 block dummy
        in_maps.append({
            "featT": featT,
            "idx": idx.reshape(NB, 1).astype(np.int32),
            "embt": emb_b,
            "WinT": WinT, "WihT": WihT, "WhhT": WhhT,
            "bias4": bias4, "stripsel": stripsel, "bin": binp,
            "ident": ident_np, "wout": wout,
        })
    _CACHE["last_in_maps"] = in_maps
    return in_maps


def _assemble_core_out(oq):
    """[5, 20, 128, 512] bf16 -> [BC, T, V] f32 for one core."""
    oq = np.asarray(oq)
    lt = oq.transpose(0, 2, 1, 3).reshape(NTW, 8, BC, VP)
    return (lt.transpose(2, 0, 1, 3).reshape(BC, T, VP)[:, :, :V]
            .astype(np.float32))


def kernel(features, seqs, lengths, W_in, b_in, emb, W_ih, W_hh, b_ih, b_hh,
           W_out, b_out):
    nc = _build()
    in_maps = _prep_in_maps(features, seqs, W_in, b_in, emb, W_ih, W_hh,
                            b_ih, b_hh, W_out)
    res = run_bass_kernel_spmd(nc, in_maps, list(range(NCORES)))
    out = np.empty((B, T, V), np.float32)
    for c in range(NCORES):
        out[c * BC:(c + 1) * BC] = _assemble_core_out(res.results[c]["out_q"])
    bo = np.asarray(b_out, dtype=np.float32)
    if np.any(bo):
        out += bo
    return out
